# revision 7
# baseline (speedup 1.0000x reference)
"""BinaryLinear (8192x4096 @ 4096x4096 binarized) on 8 TRN2 NeuronCores.

out = x @ (sign(W) * alpha).T + b

Strategy (4 row-shards for x) x (2 col-shards for W/alpha/b):
  - Per core: x rows [2048, 4096], W cols [4096, 2048]. The wide o_shard
    (2048 vs the 1x8 grid's 512) lets each stationary x tile serve 4
    moving matmuls, amortizing LDWEIGHTS 4x (266 -> 224 ns/matmul).
  - Default "hyb1792": K-columns [0, 1792) run as fp8-e4m3 DoubleRow
    matmuls (2 K-subtiles per instruction, ~2x ALU rate), the rest bf16,
    accumulating into the same PSUM banks (only the first write to a bank
    carries start=True - a start on the second half-bank write clears the
    whole bank). Rel err 1.7611e-2, bit-deterministic and HW-verified to
    match the numpy prediction exactly (gate 2e-2). Fallbacks: "hyb1536"
    (1.631e-2), "bf16" (1.66e-3, ~20% slower).
  - Weights are binarized on device: sign(W) via the ACT engine's Sign (1/3
    of slabs) and a DVE is_ge*2-1 chain (2/3), never in-place on a slice
    (that path runs ~20x slow). alpha is folded into the epilogue:
    out = psum * alpha + b, so +-1 weights are exact in both fp8 and bf16.
  - Inner loop per (nch, k): one stationary x tile, then 4 moving matmuls
    of 512 W-columns (8 of 2x256 in the fp8 phase) -> LDWEIGHTS amortized
    over 2048 streamed columns.
  - Host gathers the 8 [2048, 2048] shards into the [8192, 4096] output.

Variants: "bf16" (default, kf=0), "hyb1536", "hyb1280".
"""

import os
import sys

sys.path.insert(0, "/opt/trn_rl_repo")

import numpy as np
import ml_dtypes

from concourse import bacc, bass, mybir
import concourse.tile as tile
from concourse.bass_utils import run_bass_kernel_spmd

N_ROWS = 8192
IN_F = 4096
OUT_F = 4096
N_CORES = 8
P = 128

GRID = (4, 2)  # (row shards for x, col shards for W)
VARIANT = "hyb1792"
_KF = {"hyb1792": 1792, "hyb1536": 1536, "hyb1280": 1280, "bf16": 0}


def build_nc_hyb(
    n_shard=N_ROWS // GRID[0],
    in_f=IN_F,
    o_shard=OUT_F // GRID[1],
    alpha_one=False,
    kf=1536,
    n_chunk=128,
    o_mm=512,
    x_bufs=16,
):
    f32 = mybir.dt.float32
    bf16 = mybir.dt.bfloat16
    fp8 = mybir.dt.float8e4

    assert kf % 256 == 0 and 0 <= kf <= in_f
    KO2F = kf // 256  # fp8 DoubleRow slabs (256 K-rows each)
    KOB = (in_f - kf) // P  # bf16 slabs (128 K-rows each)
    NCH = n_shard // n_chunk
    NS = n_chunk // P
    OCH = o_shard // o_mm
    assert o_mm == 512 and NS * OCH * o_mm <= 2048

    nc = bacc.Bacc("TRN2", target_bir_lowering=False)

    xT8 = (
        nc.declare_dram_parameter("xT8", [kf, n_shard], fp8, isOutput=False)
        if kf
        else None
    )
    xTb = (
        nc.declare_dram_parameter("xTb", [in_f - kf, n_shard], bf16, isOutput=False)
        if kf < in_f
        else None
    )
    # WT holds the high byte of each bf16 weight (sign + 7 exponent MSBs):
    # sign-exact for all normal values, half the DMA of bf16.
    WT = nc.declare_dram_parameter("WT", [in_f, o_shard], mybir.dt.uint8, isOutput=False)
    a_rep = nc.declare_dram_parameter("a_rep", [P, o_shard], f32, isOutput=False)
    b_rep = nc.declare_dram_parameter("b_rep", [P, o_shard], f32, isOutput=False)
    out = nc.declare_dram_parameter("out", [n_shard, o_shard], f32, isOutput=True)

    if kf:
        xT8_t = xT8[:].rearrange("(ko2 two p) n -> ko2 p two n", two=2, p=P)
        WT8_t = WT[:].rearrange("(ko2 two p) o -> ko2 p two o", two=2, p=P)
    if kf < in_f:
        xTb_t = xTb[:].rearrange("(ko p) n -> ko p n", p=P)
        WTb_t = WT[:].rearrange("(ko p) o -> p ko o", p=P)

    if kf:
        x_bufs = min(x_bufs, 8)
    with tile.TileContext(nc) as tc:
        with (
            tc.tile_pool(name="consts", bufs=1) as consts,
            tc.tile_pool(name="wscr", bufs=2 if kf else 4) as wscrp,
            tc.tile_pool(name="wscr2", bufs=1 if kf else 2) as wscr2p,
            tc.tile_pool(name="xp", bufs=x_bufs) as xp,
            tc.tile_pool(name="outp", bufs=3) as outp,
            tc.tile_pool(name="psum", bufs=2, space="PSUM") as psump,
        ):
            bias127 = consts.tile([P, 1], f32, name="bias127")
            nc.vector.memset(bias127[:], 127.5)

            # --- weight prep: sign(W) into fp8 (k < kf) and bf16 (k >= kf)
            W_f8 = consts.tile([P, KO2F, 2, o_shard], fp8, name="W_f8") if kf else None
            for ko2 in range(KO2F):
                w2d = wscrp.tile([P, 2, o_shard], mybir.dt.uint8, tag="wscr", name="wscr")
                d_eng = nc.scalar if ko2 % 2 == 0 else nc.gpsimd
                d_eng.dma_start(out=w2d[:], in_=WT8_t[ko2])
                if ko2 % 5 == 0:
                    nc.scalar.activation(
                        W_f8[:, ko2], w2d[:],
                        mybir.ActivationFunctionType.Sign,
                        bias=bias127[:], scale=-1.0,
                    )
                else:
                    w3 = wscr2p.tile([P, 2, o_shard], bf16, tag="w3", name="w3")
                    nc.vector.tensor_scalar(
                        w3[:], w2d[:], 127.5, -2.0,
                        mybir.AluOpType.is_ge, mybir.AluOpType.mult,
                    )
                    nc.vector.tensor_scalar(
                        W_f8[:, ko2], w3[:], 1.0, None, mybir.AluOpType.add
                    )
            W_bf = (
                consts.tile([P, KOB, o_shard], bf16, name="W_bf")
                if kf < in_f
                else None
            )
            for kb in range(KOB):
                ko = kf // P + kb
                w2d = wscrp.tile([P, o_shard], mybir.dt.uint8, tag="wscrb", name="wscrb")
                d_eng = nc.scalar if kb % 2 == 0 else nc.gpsimd
                d_eng.dma_start(out=w2d[:], in_=WTb_t[:, ko])
                if kb % 5 == 0:
                    nc.scalar.activation(
                        W_bf[:, kb], w2d[:],
                        mybir.ActivationFunctionType.Sign,
                        bias=bias127[:], scale=-1.0,
                    )
                else:
                    w3 = wscr2p.tile([P, o_shard], bf16, tag="w3b", name="w3b")
                    nc.vector.tensor_scalar(
                        w3[:], w2d[:], 127.5, -2.0,
                        mybir.AluOpType.is_ge, mybir.AluOpType.mult,
                    )
                    nc.vector.tensor_scalar(
                        W_bf[:, kb], w3[:], 1.0, None, mybir.AluOpType.add
                    )

            # a/b are first needed by the nch=0 epilogue; issuing their DMA
            # after W prep keeps the scalar queue clear for W slab 0.
            a_sb = consts.tile([P, o_shard], f32, name="a_sb")
            nc.scalar.dma_start(out=a_sb[:], in_=a_rep[:])
            b_sb = consts.tile([P, o_shard], f32, name="b_sb")
            nc.scalar.dma_start(out=b_sb[:], in_=b_rep[:])

            # --- main loop
            for nch in range(NCH):
                psums = [
                    [
                        psump.tile(
                            [P, o_mm], f32,
                            tag=f"ps{ns}_{och}", name=f"ps{ns}_{och}",
                        )
                        for och in range(OCH)
                    ]
                    for ns in range(NS)
                ]
                for ko2 in range(KO2F):
                    x8_t = xp.tile([P, 2, n_chunk], fp8, tag="x8", name="x8")
                    xq_eng = nc.sync if (nch < 2 or ko2 % 2 == 0) else nc.scalar
                    xq_eng.dma_start(
                        out=x8_t[:],
                        in_=xT8_t[ko2, :, :, nch * n_chunk : (nch + 1) * n_chunk],
                    )
                    for ns in range(NS):
                        for och in range(OCH):
                            for half in range(2):
                                nc.tensor.matmul(
                                    psums[ns][och][:, half * 256 : (half + 1) * 256],
                                    x8_t[:, :, ns * P : (ns + 1) * P],
                                    W_f8[
                                        :, ko2, :,
                                        och * o_mm + half * 256 :
                                        och * o_mm + (half + 1) * 256,
                                    ],
                                    start=(ko2 == 0 and half == 0),
                                    stop=(kf == in_f and ko2 == KO2F - 1),
                                    perf_mode=mybir.MatmulPerfMode.DoubleRow,
                                    skip_group_check=True,
                                )
                for kb in range(KOB):
                    x_t = xp.tile([P, n_chunk], bf16, tag="xb", name="xb")
                    xq_eng = nc.sync if (nch < 2 or kb % 2 == 0) else nc.scalar
                    xq_eng.dma_start(
                        out=x_t[:],
                        in_=xTb_t[kb, :, nch * n_chunk : (nch + 1) * n_chunk],
                    )
                    for ns in range(NS):
                        for och in range(OCH):
                            nc.tensor.matmul(
                                psums[ns][och][:],
                                x_t[:, ns * P : (ns + 1) * P],
                                W_bf[:, kb, och * o_mm : (och + 1) * o_mm],
                                start=(kf == 0 and kb == 0),
                                stop=(kb == KOB - 1),
                                skip_group_check=True,
                            )
                for ns in range(NS):
                    o_sb = outp.tile([P, o_shard], f32, tag="o", name="o")
                    for och in range(OCH):
                        sl = slice(och * o_mm, (och + 1) * o_mm)
                        if alpha_one:
                            # alpha == 1 everywhere: out = psum + b, one op
                            nc.vector.tensor_tensor(
                                o_sb[:, sl], psums[ns][och][:], b_sb[:, sl],
                                mybir.AluOpType.add,
                            )
                        else:
                            nc.vector.tensor_tensor(
                                o_sb[:, sl], psums[ns][och][:], a_sb[:, sl],
                                mybir.AluOpType.mult,
                            )
                            nc.vector.tensor_tensor(
                                o_sb[:, sl], o_sb[:, sl], b_sb[:, sl],
                                mybir.AluOpType.add,
                            )
                    row0 = nch * n_chunk + ns * P
                    nc.gpsimd.dma_start(out=out[row0 : row0 + P, :], in_=o_sb[:])
    nc.compile()
    return nc


def make_in_maps(x, W, alpha, b, grid=GRID, kf=1536):
    """Shard full inputs into per-core input maps (host-side relayout only)."""
    xs, ws = grid
    assert xs * ws == N_CORES
    n_shard = x.shape[0] // xs
    o_shard = W.shape[0] // ws

    bf16 = ml_dtypes.bfloat16
    fp8 = ml_dtypes.float8_e4m3
    xT8 = np.ascontiguousarray(x[:, :kf].T.astype(fp8)) if kf else None
    xTb = (
        np.ascontiguousarray(x[:, kf:].T.astype(bf16)) if kf < x.shape[1] else None
    )
    in_maps = []
    w_parts = {}
    for c in range(N_CORES):
        r, q = divmod(c, ws)
        if q not in w_parts:
            sl = slice(q * o_shard, (q + 1) * o_shard)
            w_parts[q] = {
                "WT": np.ascontiguousarray(
                    (W[sl].T.astype(bf16).view(np.uint16) >> 8).astype(np.uint8)
                ),
                "a_rep": np.ascontiguousarray(
                    np.broadcast_to(alpha[sl].reshape(1, -1), (P, o_shard)),
                    dtype=np.float32,
                ),
                "b_rep": np.ascontiguousarray(
                    np.broadcast_to(b[sl].reshape(1, -1), (P, o_shard)),
                    dtype=np.float32,
                ),
            }
        m = dict(w_parts[q])
        rs = slice(r * n_shard, (r + 1) * n_shard)
        if xT8 is not None:
            m["xT8"] = np.ascontiguousarray(xT8[:, rs])
        if xTb is not None:
            m["xTb"] = np.ascontiguousarray(xTb[:, rs])
        in_maps.append(m)
    return in_maps


_NC_CACHE = {}


def kernel(x, W, alpha, b, trace=False, variant=VARIANT):
    x = np.asarray(x, dtype=np.float32)
    W = np.asarray(W, dtype=np.float32)
    alpha = np.asarray(alpha, dtype=np.float32)
    b = np.asarray(b, dtype=np.float32)

    n_rows, in_f = x.shape
    out_f = W.shape[0]
    xs, ws = GRID
    n_shard = n_rows // xs
    o_shard = out_f // ws
    kf = _KF[variant] if isinstance(variant, str) else int(variant)

    alpha_one = bool(np.all(alpha == 1.0))
    key = (n_rows, in_f, kf, alpha_one)
    if key not in _NC_CACHE:
        _NC_CACHE[key] = build_nc_hyb(
            n_shard=n_shard, in_f=in_f, o_shard=o_shard, kf=kf,
            alpha_one=alpha_one,
        )
    nc = _NC_CACHE[key]

    in_maps = make_in_maps(x, W, alpha, b, kf=kf)
    try:
        res = run_bass_kernel_spmd(
            nc, in_maps, core_ids=list(range(N_CORES)), trace=trace
        )
    except Exception:
        # The trace path needs antenv.axon_hooks + artifact upload, which
        # some containers lack. If we didn't ask for tracing ourselves,
        # retry once with tracing force-disabled instead of failing.
        if trace:
            raise
        os.environ["BASS_NEVER_TRACE"] = "1"
        res = run_bass_kernel_spmd(
            nc, in_maps, core_ids=list(range(N_CORES)), trace=False
        )
    full = np.empty((n_rows, out_f), dtype=np.float32)
    for c in range(N_CORES):
        r, q = divmod(c, ws)
        full[
            r * n_shard : (r + 1) * n_shard, q * o_shard : (q + 1) * o_shard
        ] = np.asarray(res.results[c]["out"])
    if trace:
        return full, res
    return full


if __name__ == "__main__":
    nc = build_nc_hyb(n_shard=256, in_f=1024, o_shard=1024, kf=512)
    print("build ok")
